# revision 33
# baseline (speedup 1.0000x reference)
"""Trainium2 Bass kernel for KANEX attention (MLP -> qkv -> windowed causal attention -> out proj).

The second MLP linear is folded into the qkv projection on the host:
qkv = h@Wg.T + bg = a@(Wg@W2).T + (Wg@b2 + bg) where a = silu(x@W1.T + b1),
so the device only computes one 1024x1024 layer before qkv.

Sharding: tokens are split across 8 cores for the silu layer; a is AllGathered
in two feature-halves (fp8e4m3 on the wire -- halves the collective latency,
costs ~1e-3 relative error; the qkv matmuls consume fp8 h against bf16 weights
directly); each core then computes q,k,v for its 2 heads over all tokens, runs
attention for (2 heads x 2 batches), and produces a partial output through its
128-column slice of Wo. Host sums the 8 bf16 partial outputs and adds bo.

Device-time structure (see sim.py/sim_phase.py for the local timeline sim):
- ~64 zero matmuls at t=0 keep the PE HAM activity monitor busy during the
  const DMAs so the real matmuls start at 2.4 GHz instead of 1.2 GHz.
- w1 ships o-major in 4 chunks so the first silu op starts ~6us in.
- The gathered a lands in one persistent SBUF buffer (h_sb) via direct DMAs
  from the AllGather output; qkv reads it in place (no per-pair reloads).
- The first qkv pair emits its k<4 matmuls separately so PE has ready work
  the moment AG half 0 lands (bridging the AG half 1 wait).
- Attention is software-pipelined one key-chunk deep: chunk i+1's QK matmul
  is emitted before chunk i's PV matmuls, so PE never sits behind ScalarE's
  exp. PSUM: tag "mm2" (2x 2-bank slots) holds transient psd/psv/qk tiles;
  tag "pso" (4x 1-bank slots) holds the block accumulators + fin tiles.
- keep_warm dummy matmuls bridge the AllGather waits so the PE never idles
  past the HAM re-throttle window mid-kernel.

Layout is transposed ([feature, token]) so every matmul chains without
transposes. v is computed feature-major in the same N=512 matmul loop as q/k
(og=2), then moved to [token, feature] by one 128x128 xbar DMA-transpose per
token block + a strided DVE copy that skips the ones columns (col 64 of each
65-wide block; the ones make the pv matmul also accumulate softmax sums).
The two heads' QK matmuls use disjoint 64-row groups of the PE array
(concurrent via tile_position), write the two halves of one 2-bank PSUM
tile, and share a single [128,1024] Exp.
Mask j >= i + 64 is handled by skipping fully-masked 128-key chunks; on the
5 partially-masked diagonals only the query columns that actually contain
masked keys ([off, 128*mi+64), at most 128 wide) go through a 0/1 mask-mul
on DVE, and the pv matmul splits into a masked-range and a clean-range
matmul so clean columns never wait on DVE.
"""

import numpy as np
import ml_dtypes

BF = ml_dtypes.bfloat16

N_CORES = 8
DIM = 1024
HEADS = 16
HEAD_DIM = 64
WINDOW = 64
B = 2
N = 2048
T = B * N            # 4096 tokens
TPC = T // N_CORES   # 512 tokens per core
HPC = HEADS // N_CORES  # 2 heads per core
KT = DIM // 128      # 8 k-tiles of the 1024 contraction
QB = 512             # query block (free dim)
KC = 128             # key chunk (partition dim)
NQB = N // QB        # 4 query blocks per batch
NKC = N // KC        # 16 key chunks per batch
N_WARM = 64          # PE warm-up matmuls during const DMA

LAST_RESULT = None   # BassKernelResults of the most recent run (for test harness)

_PROGRAM = None      # cached compiled Bass program
PHASE_MARKS = []     # [(label, inst_id_upper_bound)] for sim attribution


def _build_program(sim=False):
    # sim=True replaces the AllGather with plain DMA copies so the program
    # is collective-free and can run under TimelineSim (single-core timing).
    from concourse import bacc, mybir, tile

    f32 = mybir.dt.float32
    bf16 = mybir.dt.bfloat16
    f8 = mybir.dt.float8e4
    AF = mybir.ActivationFunctionType

    nc = bacc.Bacc("TRN2", target_bir_lowering=False, debug=False,
                   num_devices=N_CORES)
    PHASE_MARKS.clear()

    def mark(label):
        PHASE_MARKS.append((label, nc.next_id()))

    def inp(name, shape, dt):
        return nc.dram_tensor(name, shape, dt, kind="ExternalInput").ap()

    xT = inp("xT", [128, KT * TPC], bf16)             # [in-slice, k*512+t]
    w1 = inp("w1", [128, KT * KT * 128], bf16)        # (o,k) tiles of W1.T
    wgqk = inp("wgqk", [128, KT * 3 * 128], bf16)     # (k, og) tiles of (Wg@W2).T, og=q,k,v
    wo = inp("wo", [128, 1024], bf16)                 # [head-dim rows, 1024 out]
    b1d = inp("b1d", [128, KT], f32)
    bgqk = inp("bgqk", [128, 3], f32)
    mkd = inp("mkd", [128, 5 * 2 * 128], bf16)        # 5 diagonal 0/1 masks (x2 heads)
    out_d = nc.dram_tensor("out", [DIM, T], bf16, kind="ExternalOutput").ap()

    HALF = KT // 2 * TPC                               # feature-half of h (2048 cols)

    with tile.TileContext(nc) as tc:
        with (
            tc.tile_pool(name="const", bufs=1) as cpool,
            tc.tile_pool(name="dram", bufs=1, space="DRAM") as dpool,
            tc.tile_pool(name="att", bufs=4) as apool,
            tc.tile_pool(name="attout", bufs=6) as opool,
            tc.tile_pool(name="fin", bufs=4) as fpool,
            tc.tile_pool(name="ps", bufs=2, space="PSUM") as psP,
        ):
            # ---- persistent constants in SBUF ----
            # DMA order = SP FIFO order: what the MLP needs first loads first
            x_sb = cpool.tile([128, KT * TPC], bf16)
            nc.sync.dma_start(out=x_sb[:, :], in_=xT[:, :])
            w1_sb = cpool.tile([128, KT * KT * 128], bf16)
            for wq in range(4):                        # o-pair chunks
                c0 = wq * 2 * KT * 128
                c1 = (wq + 1) * 2 * KT * 128
                nc.sync.dma_start(out=w1_sb[:, c0:c1], in_=w1[:, c0:c1])
            b1_sb = cpool.tile([128, KT], f32)
            nc.sync.dma_start(out=b1_sb[:, :], in_=b1d[:, :])
            wgqk_sb = cpool.tile([128, KT * 3 * 128], bf16)
            nc.sync.dma_start(out=wgqk_sb[:, :], in_=wgqk[:, :])
            bgqk_sb = cpool.tile([128, 3], f32)
            nc.sync.dma_start(out=bgqk_sb[:, :], in_=bgqk[:, :])
            mk_sb = cpool.tile([128, 5 * 2 * 128], bf16)
            nc.sync.dma_start(out=mk_sb[:, :], in_=mkd[:, :])
            wo_sb = cpool.tile([128, 1024], bf16)
            nc.sync.dma_start(out=wo_sb[:, :], in_=wo[:, :])

            # ---- PE warm-up: keep the HAM activity window busy while the
            # const DMAs land, so silu starts at 2.4 GHz ----
            warm_sb = cpool.tile([128, 128], bf16)
            nc.vector.memset(warm_sb[:, :], 0.0)
            warm_ps = psP.tile([128, QB], f32, tag="pso", bufs=4, name="warm")
            for _ in range(N_WARM):
                nc.tensor.matmul(warm_ps[:, 0:128], warm_sb[:, :],
                                 warm_sb[:, :], start=True, stop=True)

            def keep_warm(n, label):
                # dummy matmuls emitted at points where the real AllGather can
                # leave PE idle past the ~3.4us HAM window: cheap insurance
                # against dropping back to the 1.2 GHz cold clock
                ps = psP.tile([128, QB], f32, tag="pso", bufs=4,
                              name=f"warm_{label}")
                for _ in range(n):
                    nc.tensor.matmul(ps[:, 0:128], warm_sb[:, :],
                                     warm_sb[:, :], start=True, stop=True)
            mark("consts")

            def w_tile_k(wsb, k, o, no):               # k-major tiling
                return wsb[:, (k * no + o) * 128:(k * no + o + 1) * 128]

            def w_tile_o(wsb, k, o):                   # o-major tiling (w1)
                return wsb[:, (o * KT + k) * 128:(o * KT + k + 1) * 128]

            # ---- silu layer on own 512 tokens (transposed layout) ----
            a_sb = cpool.tile([128, KT * TPC], f8)     # silu(x@W1.T+b1).T tiles
            a_in = [dpool.tile([128, HALF], f8, name=f"a_in{i}") for i in range(2)]
            a_space = {} if sim else {"addr_space": "Shared"}
            a_out = [dpool.tile([N_CORES, 128, HALF], f8, **a_space,
                                name=f"a_out{i}") for i in range(2)]
            # gathered h, all 8 token blocks, in SBUF for the whole kernel
            h_sb = cpool.tile([128, 8 * KT * TPC], f8)

            def h_slice(tb, k, j0=0, jn=TPC):
                base = tb * KT * TPC + k * TPC
                return h_sb[:, base + j0:base + j0 + jn]

            for half in range(2):                      # AG half 0 overlaps half 1
                for op in range(2 * half, 2 * half + 2):
                    ps = psP.tile([128, 2 * TPC], f32, tag="mm2", name=f"psL1_{op}")
                    for s in range(2):
                        o = 2 * op + s
                        for k in range(KT):
                            nc.tensor.matmul(ps[:, s * TPC:(s + 1) * TPC],
                                             w_tile_o(w1_sb, k, o),
                                             x_sb[:, k * TPC:(k + 1) * TPC],
                                             start=(k == 0), stop=(k == KT - 1))
                    for s in range(2):
                        o = 2 * op + s
                        nc.scalar.activation(a_sb[:, o * TPC:(o + 1) * TPC],
                                             ps[:, s * TPC:(s + 1) * TPC],
                                             AF.Silu, bias=b1_sb[:, o:o + 1])
                nc.sync.dma_start(out=a_in[half][:, :],
                                  in_=a_sb[:, half * HALF:(half + 1) * HALF])
                if sim:
                    for c in range(N_CORES):
                        nc.sync.dma_start(out=a_out[half][c, :, :],
                                          in_=a_in[half][:, :])
                else:
                    nc.gpsimd.collective_compute(
                        "AllGather", mybir.AluOpType.bypass,
                        replica_groups=[list(range(N_CORES))],
                        ins=[a_in[half][:, :].opt()],
                        outs=[a_out[half][:, :, :].opt()])
                # land this half of every token block into h_sb
                for tb in range(8):
                    nc.sync.dma_start(
                        out=h_sb[:, tb * KT * TPC + half * HALF:
                                 tb * KT * TPC + (half + 1) * HALF],
                        in_=a_out[half][tb, :, :])
            mark("silu")

            # ---- qkv for this core's 2 heads, all 4096 tokens ----
            # qT/kT: [128 = 2 heads x 64, 4096]; v: [token, feat] with ones cols
            qT_sb = cpool.tile([128, T], bf16)
            kT_sb = cpool.tile([128, T], bf16)
            vT_sb = cpool.tile([128, T], bf16)
            v_sb = cpool.tile([128, (T // 128) * 130], bf16)
            # whole buffer to 1.0; data writes below leave the ones columns
            # (col 64 of each 65-wide block) intact
            nc.vector.memset(v_sb[:, :], 1.0)

            def qkv_pair(tp, bridge=False):            # one token-block pair
                # q, k, vT for both blocks (3 og groups of N=512 matmuls).
                # bridge=True emits the k<4 matmuls of q/k first so PE has
                # work as soon as AG0 lands.
                dsts = [qT_sb, kT_sb, vT_sb]
                pss = {}

                def get_ps(og):
                    if og not in pss:
                        pss[og] = psP.tile([128, 2 * TPC], f32, tag="mm2",
                                           name=f"psqk_{tp}_{og}")
                    return pss[og]

                def emit_qk(og, ks):
                    ps = get_ps(og)
                    for s in range(2):
                        tb = 2 * tp + s
                        for k in ks:
                            nc.tensor.matmul(ps[:, s * TPC:(s + 1) * TPC],
                                             w_tile_k(wgqk_sb, k, og, 3),
                                             h_slice(tb, k),
                                             start=(k == 0), stop=(k == KT - 1))

                def emit_bias(og):
                    # all bias adds on DVE: ScalarE is the attention-region
                    # bottleneck (exp), so keep everything else off it
                    nc.vector.tensor_scalar_add(
                        dsts[og][:, 2 * tp * TPC:(2 * tp + 2) * TPC],
                        get_ps(og)[:, :], bgqk_sb[:, og:og + 1])

                if bridge:
                    emit_qk(0, range(KT // 2))
                    emit_qk(1, range(KT // 2))
                    keep_warm(24, f"ag1_{tp}")         # AG0 -> AG1 land
                    emit_qk(0, range(KT // 2, KT))
                    emit_bias(0)
                    emit_qk(1, range(KT // 2, KT))
                    emit_bias(1)
                else:
                    emit_qk(0, range(KT))
                    emit_bias(0)
                    emit_qk(1, range(KT))
                    emit_bias(1)
                emit_qk(2, range(KT))
                emit_bias(2)
                # v in [token, feat] via a full 128x128 xbar transpose per
                # token block (64-row transposes miscompute on HW), then a
                # strided DVE copy that skips the ones column (col 64 of
                # each 65-wide block)
                for j in range(2 * TPC // 128):
                    tj = tp * (2 * TPC // 128) + j
                    vtmp = apool.tile([128, 128], bf16, tag="vtmp",
                                      name=f"vtmp_{tj}")
                    nc.sync.dma_start(out=vtmp[:, :],
                                      in_=vT_sb[:, tj * 128:(tj + 1) * 128],
                                      transpose=True)
                    dst3 = v_sb[:, tj * 130:tj * 130 + 130].rearrange(
                        "p (h c) -> p h c", h=HPC)[:, :, 0:64]
                    src3 = vtmp[:, :].rearrange("p (h c) -> p h c", h=HPC)
                    nc.vector.tensor_copy(dst3, src3)
                mark(f"qkv{tp}")

            # ---- attention: 2 heads x 2 batches, windowed-causal ----
            # software-pipelined one chunk deep: chunk ci+1's QK is emitted
            # before chunk ci's PV, so PE runs ahead of ScalarE's exp.
            def attn_block(beta, qb):
                nch = min(4 * qb + 5, NKC)
                pso = [psP.tile([128, QB], f32, tag="pso", bufs=4,
                                name=f"psO_{beta}_{qb}_{i}")
                       for i in range(HPC)]
                attn_block.pso = pso
                q0 = beta * N + qb * QB
                pending = None                         # (pt, tj, off, w, ci)

                def emit_pv(pt, ptm, wm, tj, off, w, ci):
                    # masked cols [off, off+wm) come from ptm, clean cols
                    # [off+wm, off+w) straight from pt (no DVE dependency)
                    first = ci == 0
                    last = ci == nch - 1
                    for hh in range(HPC):
                        vsl = v_sb[:, tj * 130 + hh * 65:
                                   tj * 130 + (hh + 1) * 65]
                        if wm == 0:
                            nc.tensor.matmul(
                                pso[hh][0:65, off:off + w],
                                vsl, pt[:, hh * w:(hh + 1) * w],
                                start=first, stop=last,
                                skip_group_check=True)
                            continue
                        nc.tensor.matmul(
                            pso[hh][0:65, off:off + wm],
                            vsl, ptm[:, hh * wm:(hh + 1) * wm],
                            start=first, stop=last and wm == w,
                            skip_group_check=True)
                        if wm < w:
                            nc.tensor.matmul(
                                pso[hh][0:65, off + wm:off + w],
                                vsl, pt[:, hh * w + wm:(hh + 1) * w],
                                start=False, stop=last,
                                skip_group_check=True)

                for ci in range(nch):
                    k0 = beta * N + ci * KC
                    tj = k0 // 128
                    mi = ci - 4 * qb
                    # on diagonal chunk mi only queries f >= 128*mi-64 see
                    # any unmasked key: restrict QK/exp/PV to that range.
                    # head A packs at the end of psd bank 0, head B at the
                    # start of bank 1, so one contiguous Exp covers both.
                    off = max(0, 128 * mi - 64) if 0 <= mi <= 4 else 0
                    w = QB - off
                    psd = psP.tile([128, 2 * QB], f32, tag="mm2",
                                   name=f"psd_{beta}_{qb}_{ci}")
                    for hh in range(HPC):   # 64-row groups -> concurrent
                        col0 = QB - w if hh == 0 else QB
                        nc.tensor.matmul(
                            psd[:, col0:col0 + w],
                            kT_sb[hh * 64:(hh + 1) * 64, k0:k0 + KC],
                            qT_sb[hh * 64:(hh + 1) * 64,
                                  q0 + off:q0 + off + w],
                            start=True, stop=True)
                    pt = apool.tile([128, 2 * QB], bf16, tag="pt",
                                    bufs=6)
                    nc.scalar.activation(pt[:, 0:2 * w],
                                         psd[:, QB - w:QB + w], AF.Exp)
                    ptm = None
                    wm = 0
                    if 0 <= mi <= 4:
                        # only queries [off, 128*mi+64) contain masked keys
                        wm = min(QB, 128 * mi + 64) - off
                        ptm = apool.tile([128, 2 * 128], bf16, tag="ptm",
                                         bufs=6)
                        src3 = pt[:, 0:2 * w].rearrange(
                            "p (h c) -> p h c", h=HPC)[:, :, 0:wm]
                        msk3 = mk_sb[:, mi * 256:mi * 256 + 2 * wm].rearrange(
                            "p (h c) -> p h c", h=HPC)
                        dst3 = ptm[:, 0:2 * wm].rearrange(
                            "p (h c) -> p h c", h=HPC)
                        nc.vector.tensor_mul(dst3, src3, msk3)
                    if pending is not None:
                        emit_pv(*pending)
                    pending = (pt, ptm, wm, tj, off, w, ci)
                emit_pv(*pending)
                mark(f"attn{beta}{qb}")

            def fin_block(beta, qb, pso):
                # ---- normalize + partial out-projection, per head ----
                at = opool.tile([128, QB], bf16, tag="att",
                                name=f"att_{beta}_{qb}")
                for hh in range(HPC):
                    r = apool.tile([1, QB], f32, tag="recip")
                    nc.vector.reciprocal(r[:, :], pso[hh][64:65, :])
                    rb = apool.tile([64, QB], f32, tag="rb")
                    nc.gpsimd.partition_broadcast(rb[:, :], r[:, :])
                    nc.vector.tensor_mul(at[hh * 64:(hh + 1) * 64, :],
                                         pso[hh][0:64, :], rb[:, :])
                tcol = beta * N + qb * QB
                for og4 in range(2):                   # 4 o-slices per DMA
                    ot = fpool.tile([128, 4 * QB], bf16, tag="fin", bufs=2)
                    for oo in range(4):
                        o = og4 * 4 + oo
                        ps = psP.tile([128, QB], f32, tag="pso", bufs=4,
                                      name=f"psF_{beta}_{qb}_{o}")
                        nc.tensor.matmul(ps[:, :],
                                         wo_sb[:, o * 128:(o + 1) * 128],
                                         at[:, :], start=True, stop=True)
                        # alternate PSUM->SBUF copies between ScalarE and DVE
                        dst = ot[:, oo * QB:(oo + 1) * QB]
                        if o % 2 == 0:
                            nc.scalar.activation(dst, ps[:, :], AF.Identity)
                        else:
                            nc.vector.tensor_copy(dst, ps[:, :])
                    # one batched store: 4 x 128-row slices of out via 3D AP
                    o0 = og4 * 4
                    dst3 = out_d[o0 * 128:(o0 + 4) * 128,
                                 tcol:tcol + QB].rearrange(
                                     "(o p) t -> p o t", p=128)
                    src3 = ot[:, :].rearrange("p (o t) -> p o t", o=4)
                    nc.sync.dma_start(out=dst3, in_=src3)
                mark(f"fin{beta}{qb}")

            # interleaved emission: each attention block is emitted as soon as
            # its token blocks' q/k/v exist, so exp work reaches ScalarE
            # continuously instead of queueing behind the full qkv loop.
            # attn(b,0) needs that batch's first two token blocks only.
            # fins trail their attention block by one: the out-projection
            # then fills PE slack inside the next (ACT-bound) block's span
            def attn(beta, qb):
                attn_block(beta, qb)
                return (beta, qb, attn_block.pso)

            keep_warm(28, "ag0")                       # silu end -> AG0 land
            qkv_pair(0, bridge=True)                   # tb 0,1
            prev = attn(0, 0)
            # qkv pairs land as early as their AG halves allow, so ScalarE's
            # exp stream never drains while a pair's matmuls run; biggest
            # blocks first inside each batch, smallest block is the tail
            qkv_pair(1)                                # tb 2,3
            cur = attn(0, 3)
            fin_block(*prev); prev = cur
            qkv_pair(2)                                # tb 4,5
            cur = attn(0, 2)
            fin_block(*prev); prev = cur
            qkv_pair(3)                                # tb 6,7
            cur = attn(1, 0)
            fin_block(*prev); prev = cur
            cur = attn(0, 1)
            fin_block(*prev); prev = cur
            cur = attn(1, 3)
            fin_block(*prev); prev = cur
            cur = attn(1, 2)
            fin_block(*prev); prev = cur
            cur = attn(1, 1)
            fin_block(*prev)
            fin_block(*cur)

    nc.compile()
    return nc


def _host_prep(x, W1, b1, W2, b2, Wg, bg, Wo, bo):
    X = np.ascontiguousarray(x.reshape(T, DIM))

    def tile_wT(W_T, no, o_major=False):
        # W_T: [1024 in, no*128 out] -> [128, (k, o) tiles] (or (o, k) tiles)
        kt = W_T.shape[0] // 128
        t = W_T.reshape(kt, 128, no, 128)
        t = t.transpose(1, 2, 0, 3) if o_major else t.transpose(1, 0, 2, 3)
        return np.ascontiguousarray(t.reshape(128, kt * no * 128)).astype(BF)

    w1h = tile_wT(W1.T.astype(np.float32), KT, o_major=True)
    b1h = np.ascontiguousarray(b1.reshape(KT, 128).T).astype(np.float32)
    W2f = np.asarray(W2, dtype=np.float32)
    b2f = np.asarray(b2, dtype=np.float32)

    scale = HEAD_DIM ** -0.5
    # per-diagonal masks restricted to the live query range [off, 512), packed
    # [headA(w) | headB(w)] at column mi*1024 (matching the kernel's exp range)
    p = np.arange(128)[:, None]
    mkh_f = np.zeros((128, 5 * 2 * 128), dtype=np.float32)
    for mi in range(5):
        off = max(0, 128 * mi - 64)
        wm = min(QB, 128 * mi + 64) - off
        f = np.arange(off, off + wm)[None, :]
        m = (128 * mi + p - f <= WINDOW - 1).astype(np.float32)  # [128, wm]
        mkh_f[:, mi * 256:mi * 256 + wm] = m
        mkh_f[:, mi * 256 + wm:mi * 256 + 2 * wm] = m
    mkh = np.ascontiguousarray(mkh_f).astype(BF)

    in_maps = []
    for c in range(N_CORES):
        xc = X[c * TPC:(c + 1) * TPC].T  # [1024, 512]
        xh = np.ascontiguousarray(
            xc.reshape(KT, 128, TPC).transpose(1, 0, 2).reshape(128, KT * TPC)
        ).astype(BF)
        heads = [HPC * c + i for i in range(HPC)]
        qrows = np.concatenate([np.arange(h * 64, (h + 1) * 64) for h in heads])
        # fold W2 into the qkv projection: qkv = a @ (Wg@W2).T + (Wg@b2 + bg)
        Wg_q = np.asarray(Wg[qrows, :], np.float32) * scale
        Wg_k = np.asarray(Wg[DIM + qrows, :], np.float32)
        Wg_v = np.asarray(Wg[2 * DIM + qrows, :], np.float32)
        W2g_q = Wg_q @ W2f
        W2g_k = Wg_k @ W2f
        W2g_v = Wg_v @ W2f
        WgT_c = np.concatenate([W2g_q, W2g_k, W2g_v], axis=0).T  # [1024, 384]
        wgqkh = tile_wT(WgT_c, 3)
        bgq2 = Wg_q @ b2f + np.asarray(bg[qrows], np.float32) * scale
        bgk2 = Wg_k @ b2f + np.asarray(bg[DIM + qrows], np.float32)
        bgv2 = Wg_v @ b2f + np.asarray(bg[2 * DIM + qrows], np.float32)
        bgqkh = np.stack([bgq2, bgk2, bgv2], axis=1).astype(np.float32)
        woh = np.ascontiguousarray(Wo[:, qrows].T).astype(BF)  # [128, 1024]
        in_maps.append({
            "xT": xh, "w1": w1h, "wgqk": wgqkh,
            "wo": woh, "b1d": b1h, "bgqk": bgqkh,
            "mkd": mkh,
        })
    return in_maps


def kernel(x, W1, b1, W2, b2, Wg, bg, Wo, bo):
    global _PROGRAM, LAST_RESULT
    import os
    from concourse.bass_utils import run_bass_kernel_spmd

    if _PROGRAM is None:
        _PROGRAM = _build_program()
    in_maps = _host_prep(x, W1, b1, W2, b2, Wg, bg, Wo, bo)
    trace = bool(int(os.environ.get("KERNEL_TRACE", "0")))
    res = run_bass_kernel_spmd(_PROGRAM, in_maps, list(range(N_CORES)),
                               trace=trace)
    LAST_RESULT = res
    outT = np.zeros((DIM, T), dtype=np.float32)
    for c in range(N_CORES):
        outT += res.results[c]["out"].astype(np.float32)
    out = outT.T + bo[None, :]
    return out.reshape(B, N, DIM).astype(np.float32)


# revision 39
# speedup vs baseline: 1.4570x; 1.4570x over previous
"""Trainium2 Bass kernel for KANEX attention (MLP -> qkv -> windowed causal attention -> out proj).

The second MLP linear is folded into the qkv projection on the host:
qkv = h@Wg.T + bg = a@(Wg@W2).T + (Wg@b2 + bg) where a = silu(x@W1.T + b1),
so the device only computes one 1024x1024 layer before qkv.

Sharding: tokens are split across 8 cores for the silu layer; a is AllGathered
in two feature-halves (fp8e4m3 on the wire -- halves the collective latency,
costs ~1e-3 relative error; the qkv matmuls consume fp8 h against bf16 weights
directly); each core then computes q,k,v for its 2 heads over all tokens, runs
attention for (2 heads x 2 batches), and produces a partial output through its
128-column slice of Wo. Host sums the 8 bf16 partial outputs and adds bo.

Device-time structure (see sim.py/sim_phase.py for the local timeline sim):
- ~64 zero matmuls at t=0 keep the PE HAM activity monitor busy during the
  const DMAs so the real matmuls start at 2.4 GHz instead of 1.2 GHz.
- w1 ships o-major in 4 chunks so the first silu op starts ~6us in.
- The gathered a lands in one persistent SBUF buffer (h_sb) via direct DMAs
  from the AllGather output; qkv reads it in place (no per-pair reloads).
- The first qkv pair emits its k<4 matmuls separately so PE has ready work
  the moment AG half 0 lands (bridging the AG half 1 wait).
- Attention is software-pipelined one key-chunk deep: chunk i+1's QK matmul
  is emitted before chunk i's PV matmuls, so PE never sits behind ScalarE's
  exp. PSUM: tag "mm2" (2x 2-bank slots) holds transient psd/psv/qk tiles;
  tag "pso" (4x 1-bank slots) holds the block accumulators + fin tiles.
- keep_warm dummy matmuls bridge the AllGather waits so the PE never idles
  past the HAM re-throttle window mid-kernel.

Layout is transposed ([feature, token]) so every matmul chains without
transposes. v is computed feature-major in the same N=512 matmul loop as q/k
(og=2), then moved to [token, feature] by one 128x128 xbar DMA-transpose per
token block + a strided DVE copy that skips the ones columns (col 64 of each
65-wide block; the ones make the pv matmul also accumulate softmax sums).
The two heads' QK matmuls use disjoint 64-row groups of the PE array
(concurrent via tile_position), write the two halves of one 2-bank PSUM
tile, and share a single [128,1024] Exp.
Mask j >= i + 64 is handled by skipping fully-masked 128-key chunks; on the
5 partially-masked diagonals only the query columns that actually contain
masked keys ([off, 128*mi+64), at most 128 wide) go through a 0/1 mask-mul
on DVE, and the pv matmul splits into a masked-range and a clean-range
matmul so clean columns never wait on DVE.
"""

import numpy as np
import ml_dtypes

BF = ml_dtypes.bfloat16

N_CORES = 8
DIM = 1024
HEADS = 16
HEAD_DIM = 64
WINDOW = 64
B = 2
N = 2048
T = B * N            # 4096 tokens
TPC = T // N_CORES   # 512 tokens per core
HPC = HEADS // N_CORES  # 2 heads per core
KT = DIM // 128      # 8 k-tiles of the 1024 contraction
QB = 512             # query block (free dim)
KC = 128             # key chunk (partition dim)
NQB = N // QB        # 4 query blocks per batch
NKC = N // KC        # 16 key chunks per batch
N_WARM = 64          # PE warm-up matmuls during const DMA

LAST_RESULT = None   # BassKernelResults of the most recent run (for test harness)

_PROGRAM = None      # cached compiled Bass program
PHASE_MARKS = []     # [(label, inst_id_upper_bound)] for sim attribution


def _build_program(sim=False):
    # sim=True replaces the AllGather with plain DMA copies so the program
    # is collective-free and can run under TimelineSim (single-core timing).
    from concourse import bacc, mybir, tile

    f32 = mybir.dt.float32
    bf16 = mybir.dt.bfloat16
    f8 = mybir.dt.float8e4
    AF = mybir.ActivationFunctionType

    nc = bacc.Bacc("TRN2", target_bir_lowering=False, debug=False,
                   num_devices=N_CORES)
    PHASE_MARKS.clear()

    def mark(label):
        PHASE_MARKS.append((label, nc.next_id()))

    def inp(name, shape, dt):
        return nc.dram_tensor(name, shape, dt, kind="ExternalInput").ap()

    xT = inp("xT", [128, KT * TPC], bf16)             # [in-slice, k*512+t]
    w1 = inp("w1", [128, KT * KT * 128], bf16)        # (o,k) tiles of W1.T
    wgqk = inp("wgqk", [128, KT * 3 * 128], bf16)     # (k, og) tiles of (Wg@W2).T, og=q,k,v
    wo = inp("wo", [128, 1024], bf16)                 # [head-dim rows, 1024 out]
    b1d = inp("b1d", [128, KT], f32)
    bgqk = inp("bgqk", [128, 3], f32)
    mkd = inp("mkd", [128, 5 * 2 * 128], bf16)        # 5 diagonal 0/1 masks (x2 heads)
    out_d = nc.dram_tensor("out", [DIM, T], bf16, kind="ExternalOutput").ap()

    HALF = KT // 2 * TPC                               # feature-half of h (2048 cols)

    with tile.TileContext(nc) as tc:
        with (
            tc.tile_pool(name="const", bufs=1) as cpool,
            tc.tile_pool(name="dram", bufs=1, space="DRAM") as dpool,
            tc.tile_pool(name="att", bufs=4) as apool,
            tc.tile_pool(name="attout", bufs=6) as opool,
            tc.tile_pool(name="fin", bufs=4) as fpool,
            tc.tile_pool(name="ps", bufs=2, space="PSUM") as psP,
        ):
            # ---- persistent constants in SBUF ----
            # DMA order = SP FIFO order: what the MLP needs first loads first
            x_sb = cpool.tile([128, KT * TPC], bf16)
            nc.sync.dma_start(out=x_sb[:, :], in_=xT[:, :])
            w1_sb = cpool.tile([128, KT * KT * 128], bf16)
            for wq in range(4):                        # o-pair chunks
                c0 = wq * 2 * KT * 128
                c1 = (wq + 1) * 2 * KT * 128
                nc.sync.dma_start(out=w1_sb[:, c0:c1], in_=w1[:, c0:c1])
            b1_sb = cpool.tile([128, KT], f32)
            nc.sync.dma_start(out=b1_sb[:, :], in_=b1d[:, :])
            wgqk_sb = cpool.tile([128, KT * 3 * 128], bf16)
            nc.sync.dma_start(out=wgqk_sb[:, :], in_=wgqk[:, :])
            bgqk_sb = cpool.tile([128, 3], f32)
            nc.sync.dma_start(out=bgqk_sb[:, :], in_=bgqk[:, :])
            mk_sb = cpool.tile([128, 5 * 2 * 128], bf16)
            nc.sync.dma_start(out=mk_sb[:, :], in_=mkd[:, :])
            wo_sb = cpool.tile([128, 1024], bf16)
            nc.sync.dma_start(out=wo_sb[:, :], in_=wo[:, :])

            # ---- PE warm-up: keep the HAM activity window busy while the
            # const DMAs land, so silu starts at 2.4 GHz ----
            warm_sb = cpool.tile([128, 128], bf16)
            nc.vector.memset(warm_sb[:, :], 0.0)
            warm_ps = psP.tile([128, QB], f32, tag="pso", bufs=4, name="warm")
            for _ in range(N_WARM):
                nc.tensor.matmul(warm_ps[:, 0:128], warm_sb[:, :],
                                 warm_sb[:, :], start=True, stop=True)

            def keep_warm(n, label):
                # dummy matmuls emitted at points where the real AllGather can
                # leave PE idle past the ~3.4us HAM window: cheap insurance
                # against dropping back to the 1.2 GHz cold clock
                ps = psP.tile([128, QB], f32, tag="pso", bufs=4,
                              name=f"warm_{label}")
                for _ in range(n):
                    nc.tensor.matmul(ps[:, 0:128], warm_sb[:, :],
                                     warm_sb[:, :], start=True, stop=True)
            mark("consts")

            def w_tile_k(wsb, k, o, no):               # k-major tiling
                return wsb[:, (k * no + o) * 128:(k * no + o + 1) * 128]

            def w_tile_o(wsb, k, o):                   # o-major tiling (w1)
                return wsb[:, (o * KT + k) * 128:(o * KT + k + 1) * 128]

            # ---- silu layer on own 512 tokens (transposed layout) ----
            a_sb = cpool.tile([128, KT * TPC], f8)     # silu(x@W1.T+b1).T tiles
            a_in = [dpool.tile([128, HALF], f8, name=f"a_in{i}") for i in range(2)]
            a_space = {} if sim else {"addr_space": "Shared"}
            a_out = [dpool.tile([N_CORES, 128, HALF], f8, **a_space,
                                name=f"a_out{i}") for i in range(2)]
            # gathered h, all 8 token blocks, in SBUF for the whole kernel
            h_sb = cpool.tile([128, 8 * KT * TPC], f8)

            def h_slice(tb, k, j0=0, jn=TPC):
                base = tb * KT * TPC + k * TPC
                return h_sb[:, base + j0:base + j0 + jn]

            for half in range(2):                      # AG half 0 overlaps half 1
                for op in range(2 * half, 2 * half + 2):
                    ps = psP.tile([128, 2 * TPC], f32, tag="mm2", name=f"psL1_{op}")
                    for s in range(2):
                        o = 2 * op + s
                        for k in range(KT):
                            nc.tensor.matmul(ps[:, s * TPC:(s + 1) * TPC],
                                             w_tile_o(w1_sb, k, o),
                                             x_sb[:, k * TPC:(k + 1) * TPC],
                                             start=(k == 0), stop=(k == KT - 1))
                    for s in range(2):
                        o = 2 * op + s
                        nc.scalar.activation(a_sb[:, o * TPC:(o + 1) * TPC],
                                             ps[:, s * TPC:(s + 1) * TPC],
                                             AF.Silu, bias=b1_sb[:, o:o + 1])
                nc.sync.dma_start(out=a_in[half][:, :],
                                  in_=a_sb[:, half * HALF:(half + 1) * HALF])
                if sim:
                    for c in range(N_CORES):
                        nc.sync.dma_start(out=a_out[half][c, :, :],
                                          in_=a_in[half][:, :])
                else:
                    nc.gpsimd.collective_compute(
                        "AllGather", mybir.AluOpType.bypass,
                        replica_groups=[list(range(N_CORES))],
                        ins=[a_in[half][:, :].opt()],
                        outs=[a_out[half][:, :, :].opt()])
                # land this half of every token block into h_sb
                for tb in range(8):
                    nc.sync.dma_start(
                        out=h_sb[:, tb * KT * TPC + half * HALF:
                                 tb * KT * TPC + (half + 1) * HALF],
                        in_=a_out[half][tb, :, :])
            mark("silu")

            # ---- qkv for this core's 2 heads, all 4096 tokens ----
            # qT/kT: [128 = 2 heads x 64, 4096]; v: [token, feat] with ones cols
            qT_sb = cpool.tile([128, T], bf16)
            kT_sb = cpool.tile([128, T], bf16)
            vT_sb = cpool.tile([128, T], bf16)
            v_sb = cpool.tile([128, (T // 128) * 130], bf16)
            # whole buffer to 1.0; data writes below leave the ones columns
            # (col 64 of each 65-wide block) intact
            nc.vector.memset(v_sb[:, :], 1.0)

            def qkv_pair(tp, bridge=False):            # one token-block pair
                # q, k, vT for both blocks (3 og groups of N=512 matmuls).
                # bridge=True emits the k<4 matmuls of q/k first so PE has
                # work as soon as AG0 lands.
                dsts = [qT_sb, kT_sb, vT_sb]
                pss = {}

                def get_ps(og):
                    if og not in pss:
                        pss[og] = psP.tile([128, 2 * TPC], f32, tag="mm2",
                                           name=f"psqk_{tp}_{og}")
                    return pss[og]

                def emit_qk(og, ks):
                    ps = get_ps(og)
                    for s in range(2):
                        tb = 2 * tp + s
                        for k in ks:
                            nc.tensor.matmul(ps[:, s * TPC:(s + 1) * TPC],
                                             w_tile_k(wgqk_sb, k, og, 3),
                                             h_slice(tb, k),
                                             start=(k == 0), stop=(k == KT - 1))

                def emit_bias(og):
                    # all bias adds on DVE: ScalarE is the attention-region
                    # bottleneck (exp), so keep everything else off it
                    nc.vector.tensor_scalar_add(
                        dsts[og][:, 2 * tp * TPC:(2 * tp + 2) * TPC],
                        get_ps(og)[:, :], bgqk_sb[:, og:og + 1])

                if bridge:
                    emit_qk(0, range(KT // 2))
                    emit_qk(1, range(KT // 2))
                    keep_warm(24, f"ag1_{tp}")         # AG0 -> AG1 land
                    emit_qk(0, range(KT // 2, KT))
                    emit_bias(0)
                    emit_qk(1, range(KT // 2, KT))
                    emit_bias(1)
                else:
                    emit_qk(0, range(KT))
                    emit_bias(0)
                    emit_qk(1, range(KT))
                    emit_bias(1)
                emit_qk(2, range(KT))
                emit_bias(2)
                # v in [token, feat] via a full 128x128 xbar transpose per
                # token block (64-row transposes miscompute on HW), then a
                # strided DVE copy that skips the ones column (col 64 of
                # each 65-wide block)
                for j in range(2 * TPC // 128):
                    tj = tp * (2 * TPC // 128) + j
                    vtmp = apool.tile([128, 128], bf16, tag="vtmp",
                                      name=f"vtmp_{tj}")
                    nc.sync.dma_start(out=vtmp[:, :],
                                      in_=vT_sb[:, tj * 128:(tj + 1) * 128],
                                      transpose=True)
                    dst3 = v_sb[:, tj * 130:tj * 130 + 130].rearrange(
                        "p (h c) -> p h c", h=HPC)[:, :, 0:64]
                    src3 = vtmp[:, :].rearrange("p (h c) -> p h c", h=HPC)
                    nc.vector.tensor_copy(dst3, src3)
                mark(f"qkv{tp}")

            # ---- attention: 2 heads x 2 batches, windowed-causal ----
            # software-pipelined one chunk deep: chunk ci+1's QK is emitted
            # before chunk ci's PV, so PE runs ahead of ScalarE's exp.
            def attn_block(beta, qb):
                nch = min(4 * qb + 5, NKC)
                pso = [psP.tile([128, QB], f32, tag="pso", bufs=4,
                                name=f"psO_{beta}_{qb}_{i}")
                       for i in range(HPC)]
                attn_block.pso = pso
                q0 = beta * N + qb * QB
                pending = None                         # (pt, tj, off, w, ci)

                def emit_pv(pt, ptm, wm, tj, off, w, ci):
                    # masked cols [off, off+wm) come from ptm, clean cols
                    # [off+wm, off+w) straight from pt (no DVE dependency)
                    first = ci == 0
                    last = ci == nch - 1
                    for hh in range(HPC):
                        vsl = v_sb[:, tj * 130 + hh * 65:
                                   tj * 130 + (hh + 1) * 65]
                        if wm == 0:
                            nc.tensor.matmul(
                                pso[hh][0:65, off:off + w],
                                vsl, pt[:, hh * w:(hh + 1) * w],
                                start=first, stop=last,
                                skip_group_check=True)
                            continue
                        nc.tensor.matmul(
                            pso[hh][0:65, off:off + wm],
                            vsl, ptm[:, hh * wm:(hh + 1) * wm],
                            start=first, stop=last and wm == w,
                            skip_group_check=True)
                        if wm < w:
                            nc.tensor.matmul(
                                pso[hh][0:65, off + wm:off + w],
                                vsl, pt[:, hh * w + wm:(hh + 1) * w],
                                start=False, stop=last,
                                skip_group_check=True)

                for ci in range(nch):
                    k0 = beta * N + ci * KC
                    tj = k0 // 128
                    mi = ci - 4 * qb
                    # on diagonal chunk mi only queries f >= 128*mi-64 see
                    # any unmasked key: restrict QK/exp/PV to that range.
                    # head A packs at the end of psd bank 0, head B at the
                    # start of bank 1, so one contiguous Exp covers both.
                    off = max(0, 128 * mi - 64) if 0 <= mi <= 4 else 0
                    w = QB - off
                    psd = psP.tile([128, 2 * QB], f32, tag="mm2",
                                   name=f"psd_{beta}_{qb}_{ci}")
                    for hh in range(HPC):   # 64-row groups -> concurrent
                        col0 = QB - w if hh == 0 else QB
                        nc.tensor.matmul(
                            psd[:, col0:col0 + w],
                            kT_sb[hh * 64:(hh + 1) * 64, k0:k0 + KC],
                            qT_sb[hh * 64:(hh + 1) * 64,
                                  q0 + off:q0 + off + w],
                            start=True, stop=True)
                    pt = apool.tile([128, 2 * QB], bf16, tag="pt",
                                    bufs=6)
                    nc.scalar.activation(pt[:, 0:2 * w],
                                         psd[:, QB - w:QB + w], AF.Exp)
                    ptm = None
                    wm = 0
                    if 0 <= mi <= 4:
                        # only queries [off, 128*mi+64) contain masked keys
                        wm = min(QB, 128 * mi + 64) - off
                        ptm = apool.tile([128, 2 * 128], bf16, tag="ptm",
                                         bufs=6)
                        src3 = pt[:, 0:2 * w].rearrange(
                            "p (h c) -> p h c", h=HPC)[:, :, 0:wm]
                        msk3 = mk_sb[:, mi * 256:mi * 256 + 2 * wm].rearrange(
                            "p (h c) -> p h c", h=HPC)
                        dst3 = ptm[:, 0:2 * wm].rearrange(
                            "p (h c) -> p h c", h=HPC)
                        nc.vector.tensor_mul(dst3, src3, msk3)
                    if pending is not None:
                        emit_pv(*pending)
                    pending = (pt, ptm, wm, tj, off, w, ci)
                emit_pv(*pending)
                mark(f"attn{beta}{qb}")

            def fin_block(beta, qb, pso):
                # ---- normalize + partial out-projection, per head ----
                at = opool.tile([128, QB], bf16, tag="att",
                                name=f"att_{beta}_{qb}")
                for hh in range(HPC):
                    r = apool.tile([1, QB], f32, tag="recip")
                    nc.vector.reciprocal(r[:, :], pso[hh][64:65, :])
                    rb = apool.tile([64, QB], f32, tag="rb")
                    nc.gpsimd.partition_broadcast(rb[:, :], r[:, :])
                    nc.vector.tensor_mul(at[hh * 64:(hh + 1) * 64, :],
                                         pso[hh][0:64, :], rb[:, :])
                tcol = beta * N + qb * QB
                for og4 in range(2):                   # 4 o-slices per DMA
                    ot = fpool.tile([128, 4 * QB], bf16, tag="fin", bufs=2)
                    for oo in range(4):
                        o = og4 * 4 + oo
                        ps = psP.tile([128, QB], f32, tag="pso", bufs=4,
                                      name=f"psF_{beta}_{qb}_{o}")
                        nc.tensor.matmul(ps[:, :],
                                         wo_sb[:, o * 128:(o + 1) * 128],
                                         at[:, :], start=True, stop=True)
                        # alternate PSUM->SBUF copies between ScalarE and DVE
                        dst = ot[:, oo * QB:(oo + 1) * QB]
                        if o % 2 == 0:
                            nc.scalar.activation(dst, ps[:, :], AF.Identity)
                        else:
                            nc.vector.tensor_copy(dst, ps[:, :])
                    # one batched store: 4 x 128-row slices of out via 3D AP
                    o0 = og4 * 4
                    dst3 = out_d[o0 * 128:(o0 + 4) * 128,
                                 tcol:tcol + QB].rearrange(
                                     "(o p) t -> p o t", p=128)
                    src3 = ot[:, :].rearrange("p (o t) -> p o t", o=4)
                    nc.sync.dma_start(out=dst3, in_=src3)
                mark(f"fin{beta}{qb}")

            # interleaved emission: each attention block is emitted as soon as
            # its token blocks' q/k/v exist, so exp work reaches ScalarE
            # continuously instead of queueing behind the full qkv loop.
            # attn(b,0) needs that batch's first two token blocks only.
            # fins trail their attention block by one: the out-projection
            # then fills PE slack inside the next (ACT-bound) block's span
            def attn(beta, qb):
                attn_block(beta, qb)
                return (beta, qb, attn_block.pso)

            keep_warm(28, "ag0")                       # silu end -> AG0 land
            qkv_pair(0, bridge=True)                   # tb 0,1
            prev = attn(0, 0)
            # qkv pairs land as early as their AG halves allow, so ScalarE's
            # exp stream never drains while a pair's matmuls run; biggest
            # blocks first inside each batch, smallest block is the tail
            qkv_pair(1)                                # tb 2,3
            cur = attn(0, 3)
            fin_block(*prev); prev = cur
            cur = attn(0, 1)
            fin_block(*prev); prev = cur
            qkv_pair(2)                                # tb 4,5
            cur = attn(0, 2)
            fin_block(*prev); prev = cur
            qkv_pair(3)                                # tb 6,7
            cur = attn(1, 0)
            fin_block(*prev); prev = cur
            cur = attn(1, 3)
            fin_block(*prev); prev = cur
            cur = attn(1, 2)
            fin_block(*prev); prev = cur
            cur = attn(1, 1)
            fin_block(*prev)
            fin_block(*cur)

    nc.compile()
    return nc


def _host_prep(x, W1, b1, W2, b2, Wg, bg, Wo, bo):
    X = np.ascontiguousarray(x.reshape(T, DIM))

    def tile_wT(W_T, no, o_major=False):
        # W_T: [1024 in, no*128 out] -> [128, (k, o) tiles] (or (o, k) tiles)
        kt = W_T.shape[0] // 128
        t = W_T.reshape(kt, 128, no, 128)
        t = t.transpose(1, 2, 0, 3) if o_major else t.transpose(1, 0, 2, 3)
        return np.ascontiguousarray(t.reshape(128, kt * no * 128)).astype(BF)

    w1h = tile_wT(W1.T.astype(np.float32), KT, o_major=True)
    b1h = np.ascontiguousarray(b1.reshape(KT, 128).T).astype(np.float32)
    W2f = np.asarray(W2, dtype=np.float32)
    b2f = np.asarray(b2, dtype=np.float32)

    scale = HEAD_DIM ** -0.5
    # per-diagonal masks restricted to the live query range [off, 512), packed
    # [headA(w) | headB(w)] at column mi*1024 (matching the kernel's exp range)
    p = np.arange(128)[:, None]
    mkh_f = np.zeros((128, 5 * 2 * 128), dtype=np.float32)
    for mi in range(5):
        off = max(0, 128 * mi - 64)
        wm = min(QB, 128 * mi + 64) - off
        f = np.arange(off, off + wm)[None, :]
        m = (128 * mi + p - f <= WINDOW - 1).astype(np.float32)  # [128, wm]
        mkh_f[:, mi * 256:mi * 256 + wm] = m
        mkh_f[:, mi * 256 + wm:mi * 256 + 2 * wm] = m
    mkh = np.ascontiguousarray(mkh_f).astype(BF)

    in_maps = []
    for c in range(N_CORES):
        xc = X[c * TPC:(c + 1) * TPC].T  # [1024, 512]
        xh = np.ascontiguousarray(
            xc.reshape(KT, 128, TPC).transpose(1, 0, 2).reshape(128, KT * TPC)
        ).astype(BF)
        heads = [HPC * c + i for i in range(HPC)]
        qrows = np.concatenate([np.arange(h * 64, (h + 1) * 64) for h in heads])
        # fold W2 into the qkv projection: qkv = a @ (Wg@W2).T + (Wg@b2 + bg)
        Wg_q = np.asarray(Wg[qrows, :], np.float32) * scale
        Wg_k = np.asarray(Wg[DIM + qrows, :], np.float32)
        Wg_v = np.asarray(Wg[2 * DIM + qrows, :], np.float32)
        W2g_q = Wg_q @ W2f
        W2g_k = Wg_k @ W2f
        W2g_v = Wg_v @ W2f
        WgT_c = np.concatenate([W2g_q, W2g_k, W2g_v], axis=0).T  # [1024, 384]
        wgqkh = tile_wT(WgT_c, 3)
        bgq2 = Wg_q @ b2f + np.asarray(bg[qrows], np.float32) * scale
        bgk2 = Wg_k @ b2f + np.asarray(bg[DIM + qrows], np.float32)
        bgv2 = Wg_v @ b2f + np.asarray(bg[2 * DIM + qrows], np.float32)
        bgqkh = np.stack([bgq2, bgk2, bgv2], axis=1).astype(np.float32)
        woh = np.ascontiguousarray(Wo[:, qrows].T).astype(BF)  # [128, 1024]
        in_maps.append({
            "xT": xh, "w1": w1h, "wgqk": wgqkh,
            "wo": woh, "b1d": b1h, "bgqk": bgqkh,
            "mkd": mkh,
        })
    return in_maps


def kernel(x, W1, b1, W2, b2, Wg, bg, Wo, bo):
    global _PROGRAM, LAST_RESULT
    import os
    from concourse.bass_utils import run_bass_kernel_spmd

    if _PROGRAM is None:
        _PROGRAM = _build_program()
    in_maps = _host_prep(x, W1, b1, W2, b2, Wg, bg, Wo, bo)
    trace = bool(int(os.environ.get("KERNEL_TRACE", "0")))
    res = run_bass_kernel_spmd(_PROGRAM, in_maps, list(range(N_CORES)),
                               trace=trace)
    LAST_RESULT = res
    outT = np.zeros((DIM, T), dtype=np.float32)
    for c in range(N_CORES):
        outT += res.results[c]["out"].astype(np.float32)
    out = outT.T + bo[None, :]
    return out.reshape(B, N, DIM).astype(np.float32)
